# revision 1
# baseline (speedup 1.0000x reference)
"""Trainium2 Bass kernel for nn_HFMiMoV2DecoderLayer (attention + MoE decoder layer).

Strategy (8 NeuronCores):
  Launch 1 — tensor-parallel attention: each core owns 2 of 16 heads (and the
    matching GQA KV head), computes flash-style causal sink-softmax attention
    fully on-chip, and writes its partial Wo product [T, H].
  Host    — h1 = x + sum(partials); exact MoE routing (numpy, mirrors the
    reference); builds per-expert gathered activation matrices.
  Launch 2 — expert-parallel MoE FF: each core owns 2 of 16 experts, runs
    gate/up/silu/down on the gathered tokens, returns per-expert contributions.
  Host    — scatter-add contributions into h1.

All matmuls run as float32r (full PE rate at moving-dim >= 256, near-fp32
accuracy). The h1/routing path stays fp32 end-to-end: min routing margin for
this layer's data is ~3e-5, so bf16 anywhere before the gate would flip
top-k choices.
"""
import sys
import types

import numpy as np


def _install_ntff_hook():
    """bass_utils needs antenv.axon_hooks for NTFF tracing under axon; the
    image's antenv lacks that submodule. Inject a shim wired to the ctypes
    hook from trn_agent_boot (no-op if anything is missing)."""
    if "antenv.axon_hooks" in sys.modules:
        return
    try:
        from trn_agent_boot.trn_boot import _ntff_profile_via_ctypes

        hook = _ntff_profile_via_ctypes("/opt/axon/libaxon_pjrt.so")
    except Exception:
        hook = None
    mod = types.ModuleType("antenv.axon_hooks")
    mod._hook = hook
    mod.set_axon_ntff_profile_hook = lambda h: setattr(mod, "_hook", h)
    mod.get_axon_ntff_profile_hook = lambda: mod._hook
    sys.modules["antenv.axon_hooks"] = mod


_install_ntff_hook()

import concourse.bass as bass
import concourse.mybir as mybir
import concourse.tile as tile
from concourse import bacc
from concourse.bass_utils import run_bass_kernel_spmd
from concourse.masks import make_identity

F32 = mybir.dt.float32
F32R = mybir.dt.float32r

N_CORES = 8
T = 2048          # tokens
H = 2048          # hidden
P = 128
TCH = T // P      # 16 token chunks
HCH = H // P      # 16 hidden chunks
HD = 128          # head dim
NHC = 2           # heads per core
RD = 64           # rope dims
RH = 32
FF = 512          # moe intermediate
E = 16
EPC = 2           # experts per core
SCALE = HD ** -0.5
EPS = 1e-6
ROUTE_SCALE = 2.5
G, TG, TK = 4, 2, 4

QG = 256          # query-group width for attention
NQG = T // QG     # 8


def _r32(ap):
    return ap.bitcast(F32R)


def _mk_nc():
    return bacc.Bacc("TRN2", target_bir_lowering=False, debug=False,
                     num_devices=N_CORES)


# --------------------------------------------------------------------------
# Launch 1: attention (2 heads per core)
# --------------------------------------------------------------------------

def build_attn():
    nc = _mk_nc()
    x = nc.dram_tensor("x", [T, H], F32, kind="ExternalInput")
    wq = nc.dram_tensor("wq", [H, NHC * HD], F32R, kind="ExternalInput")
    wkv = nc.dram_tensor("wkv", [H, 2 * HD], F32R, kind="ExternalInput")
    wo = nc.dram_tensor("wo", [NHC * HD, H], F32R, kind="ExternalInput")
    cosb = nc.dram_tensor("cosb", [T, RD], F32, kind="ExternalInput")
    sinb = nc.dram_tensor("sinb", [T, RD], F32, kind="ExternalInput")
    sinke = nc.dram_tensor("sinke", [1, NHC], F32, kind="ExternalInput")
    masks = nc.dram_tensor("masks", [P, 2 * QG], F32R, kind="ExternalInput")
    partial = nc.dram_tensor("partial", [T, H], F32, kind="ExternalOutput")

    xt = x.rearrange("(tc p) h -> tc p h", p=P)
    pt_out = partial.rearrange("(tc p) h -> tc p h", p=P)

    with tile.TileContext(nc) as tc:
        with (
            tc.tile_pool(name="persist", bufs=1) as pers,
            tc.tile_pool(name="const", bufs=1) as constp,
        ):
            # persistent staging
            wq_s = pers.tile([P, HCH, NHC * HD], F32R)
            nc.sync.dma_start(wq_s[:], wq.rearrange("(hc p) n -> p hc n", p=P))
            wkv_s = pers.tile([P, HCH, 2 * HD], F32R)
            nc.sync.dma_start(wkv_s[:], wkv.rearrange("(hc p) n -> p hc n", p=P))
            wo_s = pers.tile([P, NHC, H], F32R)
            nc.sync.dma_start(wo_s[:], wo.rearrange("(h p) n -> p h n", p=P))
            cos_s = pers.tile([P, TCH, RD], F32)
            nc.sync.dma_start(cos_s[:], cosb.rearrange("(tc p) r -> p tc r", p=P))
            sin_s = pers.tile([P, TCH, RD], F32)
            nc.sync.dma_start(sin_s[:], sinb.rearrange("(tc p) r -> p tc r", p=P))
            sinke_s = pers.tile([1, NHC], F32)
            nc.sync.dma_start(sinke_s[:], sinke[:])
            mask_s = pers.tile([P, 2 * QG], F32R)
            nc.sync.dma_start(mask_s[:], masks[:])

            ident0 = constp.tile([P, P], F32)
            make_identity(nc, ident0[:])
            ident = constp.tile([P, P], F32R)
            nc.vector.tensor_copy(ident[:], ident0[:])
            ones0 = constp.tile([P, P], F32)
            nc.vector.memset(ones0[:], 1.0)
            ones_row = constp.tile([1, P], F32R)
            nc.vector.tensor_copy(ones_row[:], ones0[0:1, :])
            ones_col = constp.tile([P, 1], F32R)
            nc.vector.tensor_copy(ones_col[:], ones0[:, 0:1])

            qt_s = pers.tile([P, NHC, T], F32R)      # Q^T  [hd, h, tok]
            kt_s = pers.tile([P, T], F32R)           # K^T  [hd, tok]
            v_s = pers.tile([P, TCH, HD], F32R)      # V    [tok, tc, hd]
            ot_s = pers.tile([P, NHC, T], F32R)      # O^T  [hd, h, tok]

            # ---------------- phase A: rmsnorm + QKV + rope ----------------
            with (
                tc.tile_pool(name="xin", bufs=3) as xpool,
                tc.tile_pool(name="xn", bufs=2) as xnpool,
                tc.tile_pool(name="sq", bufs=2) as sqpool,
                tc.tile_pool(name="xnt", bufs=2) as xntpool,
                tc.tile_pool(name="qkv_sb", bufs=2) as qkvsb,
                tc.tile_pool(name="rope", bufs=2) as ropep,
                tc.tile_pool(name="stat", bufs=2) as statp,
                tc.tile_pool(name="psA", bufs=2, space="PSUM") as psA,
                tc.tile_pool(name="psT", bufs=2, space="PSUM") as psT,
            ):
                for i in range(TCH):
                    x_i = xpool.tile([P, H], F32, tag="x")
                    nc.sync.dma_start(x_i[:], xt[i])
                    sq = sqpool.tile([P, H], F32, tag="sq")
                    ssq = statp.tile([P, 1], F32, tag="ssq")
                    nc.scalar.activation(sq[:], x_i[:],
                                         mybir.ActivationFunctionType.Square,
                                         accum_out=ssq[:])
                    msq = statp.tile([P, 1], F32, tag="msq")
                    nc.vector.tensor_scalar(msq[:], ssq[:], 1.0 / H, EPS,
                                            mybir.AluOpType.mult,
                                            mybir.AluOpType.add)
                    rcp = statp.tile([P, 1], F32, tag="rcp")
                    nc.vector.reciprocal(rcp[:], msq[:])
                    r1 = statp.tile([P, 1], F32, tag="r1")
                    nc.scalar.activation(r1[:], rcp[:],
                                         mybir.ActivationFunctionType.Sqrt)
                    xn_i = xnpool.tile([P, H], F32R, tag="xn")
                    nc.scalar.activation(xn_i[:], x_i[:],
                                         mybir.ActivationFunctionType.Copy,
                                         scale=r1[:])
                    # transpose xn chunk: [tok, H] -> [H, tok] per h-chunk
                    xnt_i = xntpool.tile([P, HCH, P], F32R, tag="xnt")
                    for hc in range(HCH):
                        pst = psT.tile([P, P], F32R, tag="ptr")
                        nc.tensor.transpose(pst[:], xn_i[:, hc * P:(hc + 1) * P],
                                            ident[:])
                        nc.vector.tensor_copy(xnt_i[:, hc, :], pst[:])
                    # QKV for this token chunk
                    ps_q = psA.tile([P, NHC * HD], F32, tag="psq")
                    ps_kv = psA.tile([P, 2 * HD], F32, tag="pskv")
                    for hc in range(HCH):
                        nc.tensor.matmul(ps_q[:], _r32(xnt_i[:, hc, :]),
                                         _r32(wq_s[:, hc, :]),
                                         start=(hc == 0), stop=(hc == HCH - 1))
                        nc.tensor.matmul(ps_kv[:], _r32(xnt_i[:, hc, :]),
                                         _r32(wkv_s[:, hc, :]),
                                         start=(hc == 0), stop=(hc == HCH - 1))
                    q_sb = qkvsb.tile([P, NHC * HD], F32R, tag="qsb")
                    nc.vector.tensor_copy(q_sb[:], ps_q[:])
                    kv_sb = qkvsb.tile([P, 2 * HD], F32R, tag="kvsb")
                    nc.vector.tensor_copy(kv_sb[:], ps_kv[:])

                    # rope: heads of q, then k (offset 0 of kv)
                    cos_i = cos_s[:, i, :]
                    sin_i = sin_s[:, i, :]
                    for t_sb, offs in ((q_sb, [0, HD]), (kv_sb, [0])):
                        for ho in offs:
                            a = ropep.tile([P, RD], F32, tag="ra")
                            b = ropep.tile([P, RD], F32, tag="rb")
                            nc.vector.tensor_mul(a[:], t_sb[:, ho:ho + RD], cos_i)
                            nc.vector.tensor_mul(b[:, 0:RH],
                                                 t_sb[:, ho + RH:ho + RD],
                                                 sin_i[:, 0:RH])
                            nc.vector.tensor_mul(b[:, RH:RD],
                                                 t_sb[:, ho:ho + RH],
                                                 sin_i[:, RH:RD])
                            nc.vector.tensor_sub(t_sb[:, ho:ho + RH],
                                                 a[:, 0:RH], b[:, 0:RH])
                            nc.vector.tensor_add(t_sb[:, ho + RH:ho + RD],
                                                 a[:, RH:RD], b[:, RH:RD])
                    # transposes into QT / KT; V copied straight
                    for h in range(NHC):
                        pst = psT.tile([P, P], F32R, tag="ptr")
                        nc.tensor.transpose(pst[:], q_sb[:, h * HD:(h + 1) * HD],
                                            ident[:])
                        nc.vector.tensor_copy(qt_s[:, h, i * P:(i + 1) * P], pst[:])
                    pst = psT.tile([P, P], F32R, tag="ptr")
                    nc.tensor.transpose(pst[:], kv_sb[:, 0:HD], ident[:])
                    nc.vector.tensor_copy(kt_s[:, i * P:(i + 1) * P], pst[:])
                    nc.vector.tensor_copy(v_s[:, i, :], kv_sb[:, HD:2 * HD])

            # ---------------- phase B: attention ----------------
            with (
                tc.tile_pool(name="psS", bufs=3, space="PSUM") as psS,
                tc.tile_pool(name="psO", bufs=2, space="PSUM") as psO,
                tc.tile_pool(name="psD", bufs=2, space="PSUM") as psD,
                tc.tile_pool(name="psB", bufs=1, space="PSUM") as psB,
                tc.tile_pool(name="ptp", bufs=3) as ptp,
                tc.tile_pool(name="den", bufs=2) as denp,
            ):
                for h in range(NHC):
                    for qg in range(NQG):
                        nkt = 2 * (qg + 1)
                        q_rhs = _r32(qt_s[:, h, qg * QG:(qg + 1) * QG])
                        ps_o = psO.tile([P, QG], F32, tag="pso")
                        ps_d = psD.tile([1, QG], F32, tag="psd")
                        for kt in range(nkt):
                            ps_s = psS.tile([P, QG], F32, tag="pss")
                            nc.tensor.matmul(ps_s[:],
                                             _r32(kt_s[:, kt * P:(kt + 1) * P]),
                                             q_rhs, start=True, stop=True)
                            p_t = ptp.tile([P, QG], F32R, tag="pt")
                            nc.scalar.activation(p_t[:], ps_s[:],
                                                 mybir.ActivationFunctionType.Exp,
                                                 scale=SCALE)
                            d = kt - 2 * qg
                            if d >= 0:  # diagonal block: causal mask
                                nc.vector.tensor_mul(
                                    p_t[:], p_t[:],
                                    mask_s[:, d * QG:(d + 1) * QG])
                            nc.tensor.matmul(ps_o[:], _r32(v_s[:, kt, :]),
                                             _r32(p_t[:]),
                                             start=(kt == 0),
                                             stop=(kt == nkt - 1))
                            nc.tensor.matmul(ps_d[:], _r32(ones_col[:]),
                                             _r32(p_t[:]),
                                             start=(kt == 0),
                                             stop=(kt == nkt - 1))
                        den = denp.tile([1, QG], F32, tag="den")
                        nc.vector.tensor_scalar(den[:], ps_d[:],
                                                sinke_s[:, h:h + 1], None,
                                                mybir.AluOpType.add)
                        rec = denp.tile([1, QG], F32R, tag="rec")
                        with nc.allow_low_precision(reason="f32r rhs for PE bcast"):
                            nc.vector.reciprocal(rec[:], den[:])
                        ps_bc = psB.tile([P, QG], F32, tag="psbc")
                        nc.tensor.matmul(ps_bc[:], _r32(ones_row[:]),
                                         _r32(rec[:]), start=True, stop=True)
                        rec_bc = denp.tile([P, QG], F32, tag="recbc")
                        nc.vector.tensor_copy(rec_bc[:], ps_bc[:])
                        nc.vector.tensor_mul(ot_s[:, h, qg * QG:(qg + 1) * QG],
                                             ps_o[:], rec_bc[:])

            # ---------------- phase C: partial = O @ Wo ----------------
            with (
                tc.tile_pool(name="psW", bufs=4, space="PSUM") as psW,
                tc.tile_pool(name="outp", bufs=3) as outp,
            ):
                for ti in range(TCH):
                    out_sb = outp.tile([P, H], F32, tag="osb")
                    for nt in range(H // 512):
                        ps_p = psW.tile([P, 512], F32, tag="psp")
                        for h in range(NHC):
                            nc.tensor.matmul(
                                ps_p[:],
                                _r32(ot_s[:, h, ti * P:(ti + 1) * P]),
                                _r32(wo_s[:, h, nt * 512:(nt + 1) * 512]),
                                start=(h == 0), stop=(h == NHC - 1))
                        nc.vector.tensor_copy(out_sb[:, nt * 512:(nt + 1) * 512],
                                              ps_p[:])
                    nc.sync.dma_start(pt_out[ti], out_sb[:])

    nc.finalize()
    return nc


# --------------------------------------------------------------------------
# Launch 2: MoE expert FF (2 experts per core, capacity C tokens each)
# --------------------------------------------------------------------------

def _n_chunks(c):
    """Split c into moving-dim chunks, each in [256, 512] (c >= 256)."""
    out = []
    rem = c
    while rem > 512:
        take = 512 if rem - 512 >= 256 or rem - 512 == 0 else 384
        out.append(take)
        rem -= take
    out.append(rem)
    assert all(256 <= n <= 512 for n in out) and sum(out) == c, (c, out)
    return out


def build_moe(c_cap):
    nc = _mk_nc()
    xgt = nc.dram_tensor("xgt", [EPC, H, c_cap], F32R, kind="ExternalInput")
    wrow = nc.dram_tensor("wrow", [EPC, c_cap], F32R, kind="ExternalInput")
    weg = nc.dram_tensor("weg", [EPC, H, FF], F32R, kind="ExternalInput")
    weu = nc.dram_tensor("weu", [EPC, H, FF], F32R, kind="ExternalInput")
    wed = nc.dram_tensor("wed", [EPC, FF, H], F32R, kind="ExternalInput")
    contrib = nc.dram_tensor("contrib", [EPC * c_cap, H], F32,
                             kind="ExternalOutput")
    co = contrib.rearrange("(ec tc p) h -> ec tc p h", p=P, ec=EPC)

    nch = _n_chunks(c_cap)
    ffc_n = FF // P  # 4

    with tile.TileContext(nc) as tc:
        with (
            tc.tile_pool(name="const", bufs=1) as constp,
            tc.tile_pool(name="xg", bufs=1) as xgp,
            tc.tile_pool(name="wgu", bufs=3) as wgup,
            tc.tile_pool(name="wd", bufs=1) as wdp,
            tc.tile_pool(name="hgu", bufs=1) as hgup,
            tc.tile_pool(name="act", bufs=3) as actp,
            tc.tile_pool(name="wr", bufs=2) as wrp,
            tc.tile_pool(name="outp", bufs=3) as outp,
            tc.tile_pool(name="psG", bufs=2, space="PSUM") as psG,
            tc.tile_pool(name="psU", bufs=2, space="PSUM") as psU,
            tc.tile_pool(name="psC", bufs=3, space="PSUM") as psC,
            tc.tile_pool(name="psB", bufs=1, space="PSUM") as psB,
        ):
            ones0 = constp.tile([1, P], F32)
            nc.vector.memset(ones0[:], 1.0)
            ones_row = constp.tile([1, P], F32R)
            nc.vector.tensor_copy(ones_row[:], ones0[:])

            for e in range(EPC):
                xg_s = xgp.tile([P, HCH, c_cap], F32R, tag="xg")
                nc.sync.dma_start(xg_s[:],
                                  xgt[e].rearrange("(hc p) c -> p hc c", p=P))
                # combine-weight row broadcast to all partitions via PE
                wr_row = wrp.tile([1, c_cap], F32R, tag="wrr")
                nc.sync.dma_start(wr_row[:], wrow[e:e + 1, :])
                wbc = wrp.tile([P, c_cap], F32, tag="wbc")
                nco = 0
                for nsz in nch:
                    ps_bc = psB.tile([P, 512], F32, tag="psbc")
                    nc.tensor.matmul(ps_bc[:, :nsz], _r32(ones_row[:]),
                                     _r32(wr_row[:, nco:nco + nsz]),
                                     start=True, stop=True)
                    nc.vector.tensor_copy(wbc[:, nco:nco + nsz], ps_bc[:, :nsz])
                    nco += nsz

                wd_s = wdp.tile([P, ffc_n, H], F32R, tag="wd")
                nc.sync.dma_start(wd_s[:],
                                  wed[e].rearrange("(fc p) h -> p fc h", p=P))

                hgu = hgup.tile([P, ffc_n, c_cap], F32R, tag="hgu")
                for fc in range(ffc_n):
                    wg_s = wgup.tile([P, HCH, P], F32R, tag="wg")
                    nc.sync.dma_start(
                        wg_s[:],
                        weg[e].rearrange("(hc p) f -> p hc f", p=P)[
                            :, :, fc * P:(fc + 1) * P])
                    wu_s = wgup.tile([P, HCH, P], F32R, tag="wu")
                    nc.sync.dma_start(
                        wu_s[:],
                        weu[e].rearrange("(hc p) f -> p hc f", p=P)[
                            :, :, fc * P:(fc + 1) * P])
                    nco = 0
                    for nsz in nch:
                        ps_g = psG.tile([P, 512], F32, tag="psg")
                        ps_u = psU.tile([P, 512], F32, tag="psu")
                        for hc in range(HCH):
                            nc.tensor.matmul(ps_g[:, :nsz], _r32(wg_s[:, hc, :]),
                                             _r32(xg_s[:, hc, nco:nco + nsz]),
                                             start=(hc == 0),
                                             stop=(hc == HCH - 1))
                            nc.tensor.matmul(ps_u[:, :nsz], _r32(wu_s[:, hc, :]),
                                             _r32(xg_s[:, hc, nco:nco + nsz]),
                                             start=(hc == 0),
                                             stop=(hc == HCH - 1))
                        sg = actp.tile([P, 512], F32, tag="sg")
                        nc.scalar.activation(sg[:, :nsz], ps_g[:, :nsz],
                                             mybir.ActivationFunctionType.Silu)
                        uw = actp.tile([P, 512], F32, tag="uw")
                        nc.vector.tensor_mul(uw[:, :nsz], ps_u[:, :nsz],
                                             wbc[:, nco:nco + nsz])
                        nc.vector.tensor_mul(hgu[:, fc, nco:nco + nsz],
                                             sg[:, :nsz], uw[:, :nsz])
                        nco += nsz

                # down projection: contrib[tok, H]
                for ti in range(c_cap // P):
                    out_sb = outp.tile([P, H], F32, tag="osb")
                    for nt in range(H // 512):
                        ps_c = psC.tile([P, 512], F32, tag="psc")
                        for fc in range(ffc_n):
                            nc.tensor.matmul(
                                ps_c[:], _r32(hgu[:, fc, ti * P:(ti + 1) * P]),
                                _r32(wd_s[:, fc, nt * 512:(nt + 1) * 512]),
                                start=(fc == 0), stop=(fc == ffc_n - 1))
                        nc.vector.tensor_copy(out_sb[:, nt * 512:(nt + 1) * 512],
                                              ps_c[:])
                    nc.sync.dma_start(co[e, ti], out_sb[:])

    nc.finalize()
    return nc


# --------------------------------------------------------------------------
# Host-side routing (numpy mirror of the reference MoE gate)
# --------------------------------------------------------------------------

def _routing(h1, ln2_w, gate_w, gate_bias):
    var = np.mean(h1 * h1, axis=-1, keepdims=True)
    xf = (ln2_w * (h1 / np.sqrt(var + EPS))).astype(np.float32)
    logits = xf @ gate_w.T
    s = 1.0 / (1.0 + np.exp(-logits))
    sfc = s + gate_bias[None]
    n = sfc.shape[0]
    gview = sfc.reshape(n, G, E // G)
    gsort = np.sort(gview, axis=-1)
    group_scores = gsort[..., -1] + gsort[..., -2]
    gidx = np.argsort(-group_scores, kind="stable", axis=-1)[:, :TG]
    gmask = np.zeros((n, G), np.bool_)
    np.put_along_axis(gmask, gidx, True, axis=1)
    smask = np.repeat(gmask, E // G, axis=1)
    tmp = np.where(smask, sfc, -np.inf)
    tidx = np.argsort(-tmp, kind="stable", axis=-1)[:, :TK]
    tw = np.take_along_axis(s, tidx, axis=1)
    tw = tw / (tw.sum(-1, keepdims=True) + 1e-20)
    tw = tw * ROUTE_SCALE
    cw = np.zeros((n, E), np.float32)
    np.put_along_axis(cw, tidx, tw.astype(np.float32), axis=1)
    return xf, cw


# --------------------------------------------------------------------------
# Entry point
# --------------------------------------------------------------------------

_NC_CACHE = {}


def _get_nc(key, builder, *args):
    if key not in _NC_CACHE:
        _NC_CACHE[key] = builder(*args)
    return _NC_CACHE[key]


def kernel(hidden_states, cos, sin, ln1_w, ln2_w, Wq, Wk, Wv, Wo,
           sink_bias, gate_w, gate_bias, Weg, Weu, Wed, _profile=None):
    hidden_states, cos, sin, ln1_w, ln2_w = map(
        np.asarray, (hidden_states, cos, sin, ln1_w, ln2_w))
    Wq, Wk, Wv, Wo, sink_bias = map(np.asarray, (Wq, Wk, Wv, Wo, sink_bias))
    gate_w, gate_bias, Weg, Weu, Wed = map(
        np.asarray, (gate_w, gate_bias, Weg, Weu, Wed))
    b, s, _ = hidden_states.shape
    x = np.ascontiguousarray(hidden_states.reshape(T, H), dtype=np.float32)
    cosb = np.ascontiguousarray(cos.reshape(T, RD), dtype=np.float32)
    sinb = np.ascontiguousarray(sin.reshape(T, RD), dtype=np.float32)

    # fold ln1 into the QKV weights
    wq_f = (ln1_w[:, None] * Wq).astype(np.float32)
    wk_f = (ln1_w[:, None] * Wk).astype(np.float32)
    wv_f = (ln1_w[:, None] * Wv).astype(np.float32)

    # causal masks for the two diagonal 128x256 blocks of a query group
    kp = np.arange(P)[:, None]
    qf = np.arange(QG)[None, :]
    masks = np.concatenate([(qf >= kp), (qf >= kp + P)], axis=1).astype(np.float32)

    in_maps = []
    for c in range(N_CORES):
        h0 = NHC * c
        g0 = h0 // (16 // 4)  # kv head
        in_maps.append({
            "x": x,
            "wq": np.ascontiguousarray(wq_f[:, h0 * HD:(h0 + NHC) * HD]),
            "wkv": np.ascontiguousarray(np.concatenate(
                [wk_f[:, g0 * HD:(g0 + 1) * HD],
                 wv_f[:, g0 * HD:(g0 + 1) * HD]], axis=1)),
            "wo": np.ascontiguousarray(Wo[h0 * HD:(h0 + NHC) * HD, :]),
            "cosb": cosb,
            "sinb": sinb,
            "sinke": np.exp(sink_bias[h0:h0 + NHC]).reshape(1, NHC)
                       .astype(np.float32),
            "masks": masks,
        })

    nc1 = _get_nc("attn", build_attn)
    res1 = run_bass_kernel_spmd(nc1, in_maps, core_ids=list(range(N_CORES)),
                                trace=_profile is not None)
    h1 = x.copy()
    for c in range(N_CORES):
        h1 += res1.results[c]["partial"]

    xf, cw = _routing(h1, np.asarray(ln2_w), np.asarray(gate_w),
                      np.asarray(gate_bias))

    idxs = [np.nonzero(cw[:, e] > 0)[0] for e in range(E)]
    maxc = max(len(ix) for ix in idxs)
    c_cap = max(512, -(-maxc // P) * P)

    in_maps2 = []
    for c in range(N_CORES):
        xg = np.zeros((EPC, H, c_cap), np.float32)
        wr = np.zeros((EPC, c_cap), np.float32)
        for j in range(EPC):
            e = EPC * c + j
            ix = idxs[e]
            xg[j, :, :len(ix)] = xf[ix].T
            wr[j, :len(ix)] = cw[ix, e]
        in_maps2.append({
            "xgt": xg,
            "wrow": wr,
            "weg": np.ascontiguousarray(Weg[EPC * c:EPC * (c + 1)],
                                        dtype=np.float32),
            "weu": np.ascontiguousarray(Weu[EPC * c:EPC * (c + 1)],
                                        dtype=np.float32),
            "wed": np.ascontiguousarray(Wed[EPC * c:EPC * (c + 1)],
                                        dtype=np.float32),
        })

    nc2 = _get_nc(("moe", c_cap), build_moe, c_cap)
    res2 = run_bass_kernel_spmd(nc2, in_maps2, core_ids=list(range(N_CORES)),
                                trace=_profile is not None)

    out = h1
    for c in range(N_CORES):
        cb = res2.results[c]["contrib"].reshape(EPC, c_cap, H)
        for j in range(EPC):
            e = EPC * c + j
            ix = idxs[e]
            out[ix] += cb[j, :len(ix)]

    if _profile is not None:
        _profile["attn_ns"] = res1.exec_time_ns
        _profile["moe_ns"] = res2.exec_time_ns
        _profile["res1"] = res1
        _profile["res2"] = res2

    return out.reshape(b, s, H)



# revision 7
# speedup vs baseline: 1.3438x; 1.3438x over previous
"""Trainium2 Bass kernel for nn_HFMiMoV2DecoderLayer (attention + MoE decoder layer).

Strategy (8 NeuronCores):
  Launch 1 — tensor-parallel attention: each core owns 2 of 16 heads (and the
    matching GQA KV head). Host pre-transposes x and pre-computes per-token
    RMS scales, so the device does no rmsnorm/transpose of the full x.
    QKV is one merged [H, 512] matmul per core; rope is 4 strided-AP vector
    ops per token chunk; flash-style causal sink-softmax with ln/exp
    denominator; partial Wo product [T, H] written per core.
  Host    — h1 = x + sum(partials); exact MoE routing (numpy, mirrors the
    reference); builds per-expert gathered activation matrices.
  Launch 2 — expert-parallel MoE FF in bf16 (post-gate path is precision-
    safe): each core owns 2 of 16 experts; combine weight folded into the
    PSUM->SBUF output copy.
  Host    — scatter-add contributions into h1.

The h1/routing path stays fp32 end-to-end: min routing margin for this
layer's data is ~3e-5; perturbation tests show bf16 anywhere before the gate
risks a top-k flip costing ~1.4e-1 rel err. Post-gate bf16 measures 1.2e-3.
"""
import sys
import types

import numpy as np


def _install_ntff_hook():
    """bass_utils needs antenv.axon_hooks for NTFF tracing under axon; the
    image's antenv lacks that submodule. Inject a shim wired to the ctypes
    hook from trn_agent_boot (no-op if anything is missing)."""
    if "antenv.axon_hooks" in sys.modules:
        return
    try:
        from trn_agent_boot.trn_boot import _ntff_profile_via_ctypes

        hook = _ntff_profile_via_ctypes("/opt/axon/libaxon_pjrt.so")
    except Exception:
        hook = None
    mod = types.ModuleType("antenv.axon_hooks")
    mod._hook = hook
    mod.set_axon_ntff_profile_hook = lambda h: setattr(mod, "_hook", h)
    mod.get_axon_ntff_profile_hook = lambda: mod._hook
    sys.modules["antenv.axon_hooks"] = mod


_install_ntff_hook()

import ml_dtypes

import concourse.bass as bass
import concourse.mybir as mybir
import concourse.tile as tile
from concourse import bacc
from concourse.bass_utils import run_bass_kernel_spmd
from concourse.masks import make_identity

F32 = mybir.dt.float32
F32R = mybir.dt.float32r
BF16 = mybir.dt.bfloat16
BF = ml_dtypes.bfloat16

N_CORES = 8
T = 2048          # tokens
H = 2048          # hidden
P = 128
TCH = T // P      # 16 token chunks
HCH = H // P      # 16 hidden chunks
HD = 128          # head dim
NHC = 2           # heads per core
RD = 64           # rope dims
RH = 32
FF = 512          # moe intermediate
FFC = FF // P     # 4
E = 16
EPC = 2           # experts per core
SCALE = HD ** -0.5
EPS = 1e-6
ROUTE_SCALE = 2.5
G, TG, TK = 4, 2, 4

QG = 512          # query-group width for attention
NQG = T // QG     # 4
TGRP = 4          # token chunks loaded per DMA in phase A


def _r32(ap):
    return ap.bitcast(F32R)


def _mk_nc():
    return bacc.Bacc("TRN2", target_bir_lowering=False, debug=False,
                     num_devices=N_CORES)


# --------------------------------------------------------------------------
# Launch 1: attention (2 heads per core)
# --------------------------------------------------------------------------

def build_attn():
    nc = _mk_nc()
    xt = nc.dram_tensor("xt", [H, T], F32R, kind="ExternalInput")
    wqkv = nc.dram_tensor("wqkv", [H, 4 * P], F32R, kind="ExternalInput")
    rsc = nc.dram_tensor("rsc", [P, TCH], F32, kind="ExternalInput")
    cosr = nc.dram_tensor("cosr", [P, TCH, RD], F32, kind="ExternalInput")
    sinr = nc.dram_tensor("sinr", [P, TCH, RD], F32, kind="ExternalInput")
    sinke = nc.dram_tensor("sinke", [1, NHC], F32, kind="ExternalInput")
    wo = nc.dram_tensor("wo", [NHC * HD, H], F32R, kind="ExternalInput")
    partial = nc.dram_tensor("partial", [T, H], F32, kind="ExternalOutput")

    xt_r = xt.rearrange("(hc p) t -> p hc t", p=P)
    pt_out = partial.rearrange("(tc p) h -> tc p h", p=P)

    with tile.TileContext(nc) as tc:
        with (
            tc.tile_pool(name="persist", bufs=1) as pers,
            tc.tile_pool(name="const", bufs=1) as constp,
        ):
            wqkv_s = pers.tile([P, HCH, 4 * P], F32R)
            nc.sync.dma_start(wqkv_s[:], wqkv.rearrange("(hc p) n -> p hc n", p=P))
            rsc_s = pers.tile([P, TCH], F32)
            nc.sync.dma_start(rsc_s[:], rsc[:])
            cos_s = pers.tile([P, TCH, RD], F32)
            nc.sync.dma_start(cos_s[:], cosr[:])
            sin_s = pers.tile([P, TCH, RD], F32)
            nc.sync.dma_start(sin_s[:], sinr[:])
            sinke_s = pers.tile([1, NHC], F32)
            nc.sync.dma_start(sinke_s[:], sinke[:])

            ident0 = constp.tile([P, P], F32)
            make_identity(nc, ident0[:])
            ident = constp.tile([P, P], F32R)
            nc.vector.tensor_copy(ident[:], ident0[:])
            ones0 = constp.tile([P, P], F32)
            nc.vector.memset(ones0[:], 1.0)
            ones_row = constp.tile([1, P], F32R)
            nc.vector.tensor_copy(ones_row[:], ones0[0:1, :])
            ones_col = constp.tile([P, 1], F32R)
            nc.vector.tensor_copy(ones_col[:], ones0[:, 0:1])

            # diagonal-block causal masks, built on the idle gpsimd engine:
            # mask[p, d, q] = 1.0 if q >= 128*d + p else 0.0
            mask_s = constp.tile([P, NQG, QG], F32)
            nc.gpsimd.memset(mask_s[:], 1.0)
            for d in range(4):
                nc.gpsimd.affine_select(
                    out=mask_s[:, d, :], in_=mask_s[:, d, :],
                    compare_op=mybir.AluOpType.is_ge, fill=0.0,
                    base=-(P * d), pattern=[[1, QG]], channel_multiplier=-1)

            qkv_sb = pers.tile([P, TCH, 4 * P], F32R)  # roped q0|q1|k|v per chunk
            qkt_s = pers.tile([P, 3, T], F32R)         # q0^T | q1^T | k^T
            ot_s = pers.tile([P, NHC, T], F32R)        # O^T  [hd, h, tok]

            # ---------------- phase A: QKV + rope + transposes ----------------
            with (
                tc.tile_pool(name="xin", bufs=2) as xpool,
                tc.tile_pool(name="rope", bufs=2) as ropep,
                tc.tile_pool(name="psA", bufs=2, space="PSUM") as psA,
                tc.tile_pool(name="psT", bufs=2, space="PSUM") as psT,
            ):
                for tg in range(TCH // TGRP):
                    xt_g = xpool.tile([P, HCH, TGRP * P], F32R, tag="xt")
                    nc.sync.dma_start(
                        xt_g[:], xt_r[:, :, tg * TGRP * P:(tg + 1) * TGRP * P])
                    for j in range(TGRP):
                        tcx = tg * TGRP + j
                        ts = slice(j * P, (j + 1) * P)
                        ps_qkv = psA.tile([P, 4 * P], F32, tag="qkv")
                        for hc in range(HCH):
                            nc.tensor.matmul(ps_qkv[:], _r32(xt_g[:, hc, ts]),
                                             _r32(wqkv_s[:, hc, :]),
                                             start=(hc == 0), stop=(hc == HCH - 1))
                        qtc = qkv_sb[:, tcx, :]
                        nc.scalar.activation(qtc, ps_qkv[:],
                                             mybir.ActivationFunctionType.Copy,
                                             scale=rsc_s[:, tcx:tcx + 1])
                        # rope on q0, q1, k via strided views [P, 3, 64]
                        qv = qtc.rearrange("p (b c) -> p b c", c=P)
                        ro = qv[:, 0:3, 0:RD]
                        cos_b = cos_s[:, tcx:tcx + 1, :].broadcast_to([P, 3, RD])
                        sin_lo = sin_s[:, tcx:tcx + 1, 0:RH].broadcast_to([P, 3, RH])
                        sin_hi = sin_s[:, tcx:tcx + 1, RH:RD].broadcast_to([P, 3, RH])
                        b = ropep.tile([P, 3, RD], F32, tag="rb")
                        nc.vector.tensor_mul(b[:, :, 0:RH], qv[:, 0:3, RH:RD],
                                             sin_lo)
                        nc.vector.tensor_mul(b[:, :, RH:RD], qv[:, 0:3, 0:RH],
                                             sin_hi)
                        a = ropep.tile([P, 3, RD], F32, tag="ra")
                        nc.vector.tensor_mul(a[:], ro, cos_b)
                        nc.vector.tensor_add(ro, a[:], b[:])
                        # transpose q0, q1, k into qkt_s
                        ps_t = psT.tile([P, 3, P], F32R, tag="pt")
                        for i in range(3):
                            nc.tensor.transpose(ps_t[:, i, :], qv[:, i, :],
                                                ident[:])
                        nc.vector.tensor_copy(
                            qkt_s[:, :, tcx * P:(tcx + 1) * P], ps_t[:])

            # ---------------- phase B + C interleaved ----------------
            with (
                tc.tile_pool(name="wop", bufs=1) as wop,
                tc.tile_pool(name="psS", bufs=3, space="PSUM") as psS,
                tc.tile_pool(name="psO", bufs=2, space="PSUM") as psO,
                tc.tile_pool(name="psD", bufs=1, space="PSUM") as psD,
                tc.tile_pool(name="psB", bufs=1, space="PSUM") as psB,
                tc.tile_pool(name="psW", bufs=1, space="PSUM") as psW,
                tc.tile_pool(name="ptp", bufs=3) as ptp,
                tc.tile_pool(name="den", bufs=2) as denp,
                tc.tile_pool(name="outp", bufs=3) as outp,
            ):
                wo_s = wop.tile([P, NHC, H], F32R)
                nc.sync.dma_start(wo_s[:], wo.rearrange("(h p) n -> p h n", p=P))

                for qg in range(NQG):
                    nkt = 4 * (qg + 1)
                    for h in range(NHC):
                        q_rhs = _r32(qkt_s[:, h, qg * QG:(qg + 1) * QG])
                        ps_o = psO.tile([P, QG], F32, tag="pso")
                        ps_d = psD.tile([1, QG], F32, tag="psd")
                        for kt in range(nkt):
                            ps_s = psS.tile([P, QG], F32, tag="pss")
                            nc.tensor.matmul(
                                ps_s[:], _r32(qkt_s[:, 2, kt * P:(kt + 1) * P]),
                                q_rhs, start=True, stop=True)
                            p_t = ptp.tile([P, QG], F32R, tag="pt")
                            nc.scalar.activation(
                                p_t[:], ps_s[:],
                                mybir.ActivationFunctionType.Exp, scale=SCALE)
                            d = kt - 4 * qg
                            if d >= 0:  # diagonal block: causal mask
                                nc.vector.tensor_mul(p_t[:], p_t[:],
                                                     mask_s[:, d, :])
                            nc.tensor.matmul(
                                ps_o[:], _r32(qkv_sb[:, kt, 3 * P:4 * P]),
                                _r32(p_t[:]),
                                start=(kt == 0), stop=(kt == nkt - 1))
                            nc.tensor.matmul(
                                ps_d[:], ones_col[:], _r32(p_t[:]),
                                start=(kt == 0), stop=(kt == nkt - 1))
                        # 1/den via ln+exp (same activation table set as Exp)
                        lnd = denp.tile([1, QG], F32, tag="lnd")
                        nc.scalar.activation(lnd[:], ps_d[:],
                                             mybir.ActivationFunctionType.Ln,
                                             bias=sinke_s[0:1, h:h + 1])
                        rec = denp.tile([1, QG], F32R, tag="rec")
                        nc.scalar.activation(rec[:], lnd[:],
                                             mybir.ActivationFunctionType.Exp,
                                             scale=-1.0)
                        ps_bc = psB.tile([P, QG], F32, tag="psbc")
                        nc.tensor.matmul(ps_bc[:], ones_row[:], _r32(rec[:]),
                                         start=True, stop=True)
                        rbc = denp.tile([P, QG], F32, tag="rbc")
                        nc.vector.tensor_copy(rbc[:], ps_bc[:])
                        nc.vector.tensor_mul(ot_s[:, h, qg * QG:(qg + 1) * QG],
                                             ps_o[:], rbc[:])
                    # phase C for the token chunks this query group completed
                    for ti in range(4 * qg, 4 * (qg + 1)):
                        out_sb = outp.tile([P, H], F32, tag="osb")
                        for nt in range(H // 512):
                            ps_p = psW.tile([P, 512], F32, tag="psw")
                            for h in range(NHC):
                                nc.tensor.matmul(
                                    ps_p[:],
                                    _r32(ot_s[:, h, ti * P:(ti + 1) * P]),
                                    _r32(wo_s[:, h, nt * 512:(nt + 1) * 512]),
                                    start=(h == 0), stop=(h == NHC - 1))
                            dst = out_sb[:, nt * 512:(nt + 1) * 512]
                            if nt % 2 == 0:
                                nc.scalar.activation(
                                    dst, ps_p[:],
                                    mybir.ActivationFunctionType.Copy)
                            else:
                                nc.vector.tensor_copy(dst, ps_p[:])
                        nc.sync.dma_start(pt_out[ti], out_sb[:])

    nc.finalize()
    return nc


# --------------------------------------------------------------------------
# Launch 2: MoE expert FF in bf16 (2 experts per core, capacity c_cap each)
# --------------------------------------------------------------------------

def _n_chunks(c):
    """Split c into moving-dim chunks, each in [256, 512] (c >= 512)."""
    out = []
    rem = c
    while rem > 512:
        take = 512 if rem - 512 >= 256 or rem - 512 == 0 else 384
        out.append(take)
        rem -= take
    out.append(rem)
    assert all(128 <= n <= 512 for n in out) and sum(out) == c, (c, out)
    return out


def build_moe(c_cap):
    nc = _mk_nc()
    cch = c_cap // P
    xgt = nc.dram_tensor("xgt", [EPC, H, c_cap], BF16, kind="ExternalInput")
    wrow = nc.dram_tensor("wrow", [P, EPC, cch], F32, kind="ExternalInput")
    weg = nc.dram_tensor("weg", [EPC, H, FF], BF16, kind="ExternalInput")
    weu = nc.dram_tensor("weu", [EPC, H, FF], BF16, kind="ExternalInput")
    wed = nc.dram_tensor("wed", [EPC, FF, H], BF16, kind="ExternalInput")
    contrib = nc.dram_tensor("contrib", [EPC * c_cap, H], BF16,
                             kind="ExternalOutput")
    co = contrib.rearrange("(ec tc p) h -> ec tc p h", p=P, ec=EPC)

    nch = _n_chunks(c_cap)
    ncn = len(nch)

    with tile.TileContext(nc) as tc:
        with (
            tc.tile_pool(name="wr", bufs=1) as wrp,
            tc.tile_pool(name="xg", bufs=2) as xgp,
            tc.tile_pool(name="wgu", bufs=6) as wgup,
            tc.tile_pool(name="wd", bufs=2) as wdp,
            tc.tile_pool(name="hgu", bufs=2) as hgup,
            tc.tile_pool(name="act", bufs=3) as actp,
            tc.tile_pool(name="outp", bufs=3) as outp,
            tc.tile_pool(name="psGU", bufs=1, space="PSUM") as psGU,
            tc.tile_pool(name="psC", bufs=2, space="PSUM") as psC,
        ):
            wr_s = wrp.tile([P, EPC, cch], F32)
            nc.sync.dma_start(wr_s[:], wrow[:])

            for e in range(EPC):
                xg_s = xgp.tile([P, HCH, c_cap], BF16, tag="xg")
                nc.sync.dma_start(xg_s[:],
                                  xgt[e].rearrange("(hc p) c -> p hc c", p=P))
                wd_s = wdp.tile([P, FFC, H], BF16, tag="wd")
                nc.sync.dma_start(wd_s[:],
                                  wed[e].rearrange("(fc p) h -> p fc h", p=P))

                hgu = hgup.tile([P, FFC, c_cap], BF16, tag="hgu")
                for fc in range(FFC):
                    wg_s = wgup.tile([P, HCH, P], BF16, tag="wg")
                    nc.sync.dma_start(
                        wg_s[:],
                        weg[e].rearrange("(hc p) f -> p hc f", p=P)[
                            :, :, fc * P:(fc + 1) * P])
                    wu_s = wgup.tile([P, HCH, P], BF16, tag="wu")
                    nc.sync.dma_start(
                        wu_s[:],
                        weu[e].rearrange("(hc p) f -> p hc f", p=P)[
                            :, :, fc * P:(fc + 1) * P])
                    ps_gs = [psGU.tile([P, 512], F32, tag=f"psg{j}",
                                       name=f"ps_g{j}") for j in range(ncn)]
                    ps_us = [psGU.tile([P, 512], F32, tag=f"psu{j}",
                                       name=f"ps_u{j}") for j in range(ncn)]
                    for hc in range(HCH):
                        nco = 0
                        for j, nsz in enumerate(nch):
                            nc.tensor.matmul(ps_gs[j][:, :nsz], wg_s[:, hc, :],
                                             xg_s[:, hc, nco:nco + nsz],
                                             start=(hc == 0),
                                             stop=(hc == HCH - 1))
                            nc.tensor.matmul(ps_us[j][:, :nsz], wu_s[:, hc, :],
                                             xg_s[:, hc, nco:nco + nsz],
                                             start=(hc == 0),
                                             stop=(hc == HCH - 1))
                            nco += nsz
                    nco = 0
                    for j, nsz in enumerate(nch):
                        sg = actp.tile([P, 512], F32, tag="sg")
                        nc.scalar.activation(sg[:, :nsz], ps_gs[j][:, :nsz],
                                             mybir.ActivationFunctionType.Silu)
                        nc.vector.tensor_mul(hgu[:, fc, nco:nco + nsz],
                                             sg[:, :nsz], ps_us[j][:, :nsz])
                        nco += nsz

                # down projection, combine weight folded into the output copy
                for ti in range(cch):
                    out_sb = outp.tile([P, H], BF16, tag="osb")
                    wr_ap = wr_s[:, e, ti:ti + 1]
                    for ntg in range(2):
                        ps_c0 = psC.tile([P, 512], F32, tag="psc0")
                        ps_c1 = psC.tile([P, 512], F32, tag="psc1")
                        for fc in range(FFC):
                            for k, ps_c in enumerate((ps_c0, ps_c1)):
                                nt = 2 * ntg + k
                                nc.tensor.matmul(
                                    ps_c[:],
                                    hgu[:, fc, ti * P:(ti + 1) * P],
                                    wd_s[:, fc, nt * 512:(nt + 1) * 512],
                                    start=(fc == 0), stop=(fc == FFC - 1))
                        for k, ps_c in enumerate((ps_c0, ps_c1)):
                            nt = 2 * ntg + k
                            dst = out_sb[:, nt * 512:(nt + 1) * 512]
                            if k == 0:
                                nc.scalar.activation(
                                    dst, ps_c[:],
                                    mybir.ActivationFunctionType.Copy,
                                    scale=wr_ap)
                            else:
                                nc.vector.tensor_scalar(
                                    dst, ps_c[:], wr_ap, None,
                                    mybir.AluOpType.mult)
                    nc.sync.dma_start(co[e, ti], out_sb[:])

    nc.finalize()
    return nc


# --------------------------------------------------------------------------
# Host-side routing (numpy mirror of the reference MoE gate)
# --------------------------------------------------------------------------

def _routing(h1, ln2_w, gate_w, gate_bias):
    var = np.mean(h1 * h1, axis=-1, keepdims=True)
    xf = (ln2_w * (h1 / np.sqrt(var + EPS))).astype(np.float32)
    logits = xf @ gate_w.T
    s = 1.0 / (1.0 + np.exp(-logits))
    sfc = s + gate_bias[None]
    n = sfc.shape[0]
    gview = sfc.reshape(n, G, E // G)
    gsort = np.sort(gview, axis=-1)
    group_scores = gsort[..., -1] + gsort[..., -2]
    gidx = np.argsort(-group_scores, kind="stable", axis=-1)[:, :TG]
    gmask = np.zeros((n, G), np.bool_)
    np.put_along_axis(gmask, gidx, True, axis=1)
    smask = np.repeat(gmask, E // G, axis=1)
    tmp = np.where(smask, sfc, -np.inf)
    tidx = np.argsort(-tmp, kind="stable", axis=-1)[:, :TK]
    tw = np.take_along_axis(s, tidx, axis=1)
    tw = tw / (tw.sum(-1, keepdims=True) + 1e-20)
    tw = tw * ROUTE_SCALE
    cw = np.zeros((n, E), np.float32)
    np.put_along_axis(cw, tidx, tw.astype(np.float32), axis=1)
    return xf, cw


# --------------------------------------------------------------------------
# Entry point
# --------------------------------------------------------------------------

_NC_CACHE = {}


def _get_nc(key, builder, *args):
    if key not in _NC_CACHE:
        _NC_CACHE[key] = builder(*args)
    return _NC_CACHE[key]


def kernel(hidden_states, cos, sin, ln1_w, ln2_w, Wq, Wk, Wv, Wo,
           sink_bias, gate_w, gate_bias, Weg, Weu, Wed, _profile=None):
    hidden_states, cos, sin, ln1_w, ln2_w = map(
        np.asarray, (hidden_states, cos, sin, ln1_w, ln2_w))
    Wq, Wk, Wv, Wo, sink_bias = map(np.asarray, (Wq, Wk, Wv, Wo, sink_bias))
    gate_w, gate_bias, Weg, Weu, Wed = map(
        np.asarray, (gate_w, gate_bias, Weg, Weu, Wed))
    b, s, _ = hidden_states.shape
    x = np.ascontiguousarray(hidden_states.reshape(T, H), dtype=np.float32)
    cosb = np.ascontiguousarray(cos.reshape(T, RD), dtype=np.float32)
    sinb = np.ascontiguousarray(sin.reshape(T, RD), dtype=np.float32)

    # host-side prep: transpose of x, per-token 1/rms, rope tables
    xt = np.ascontiguousarray(x.T)
    r = (1.0 / np.sqrt((x * x).mean(-1) + EPS)).astype(np.float32)
    rsc = np.ascontiguousarray(r.reshape(TCH, P).T)
    cosr = np.ascontiguousarray(cosb.reshape(TCH, P, RD).transpose(1, 0, 2))
    ss = sinb.copy()
    ss[:, :RH] *= -1.0
    sinr = np.ascontiguousarray(ss.reshape(TCH, P, RD).transpose(1, 0, 2))

    # fold ln1 into the QKV weights
    wq_f = (ln1_w[:, None] * Wq).astype(np.float32)
    wk_f = (ln1_w[:, None] * Wk).astype(np.float32)
    wv_f = (ln1_w[:, None] * Wv).astype(np.float32)

    in_maps = []
    for c in range(N_CORES):
        h0 = NHC * c
        g0 = h0 // (16 // 4)  # kv head
        in_maps.append({
            "xt": xt,
            "wqkv": np.ascontiguousarray(np.concatenate(
                [wq_f[:, h0 * HD:(h0 + NHC) * HD],
                 wk_f[:, g0 * HD:(g0 + 1) * HD],
                 wv_f[:, g0 * HD:(g0 + 1) * HD]], axis=1)),
            "rsc": rsc,
            "cosr": cosr,
            "sinr": sinr,
            "sinke": np.exp(sink_bias[h0:h0 + NHC]).reshape(1, NHC)
                       .astype(np.float32),
            "wo": np.ascontiguousarray(Wo[h0 * HD:(h0 + NHC) * HD, :]),
        })

    nc1 = _get_nc("attn", build_attn)
    res1 = run_bass_kernel_spmd(nc1, in_maps, core_ids=list(range(N_CORES)),
                                trace=_profile is not None)
    h1 = x.copy()
    for c in range(N_CORES):
        h1 += res1.results[c]["partial"]

    xf, cw = _routing(h1, np.asarray(ln2_w), np.asarray(gate_w),
                      np.asarray(gate_bias))

    idxs = [np.nonzero(cw[:, e] > 0)[0] for e in range(E)]
    maxc = max(len(ix) for ix in idxs)
    c_cap = max(512, -(-maxc // P) * P)
    cch = c_cap // P

    in_maps2 = []
    for c in range(N_CORES):
        xg = np.zeros((EPC, H, c_cap), BF)
        wr = np.zeros((EPC, c_cap), np.float32)
        for j in range(EPC):
            e = EPC * c + j
            ix = idxs[e]
            xg[j, :, :len(ix)] = xf[ix].T.astype(BF)
            wr[j, :len(ix)] = cw[ix, e]
        in_maps2.append({
            "xgt": xg,
            "wrow": np.ascontiguousarray(
                wr.reshape(EPC, cch, P).transpose(2, 0, 1)),
            "weg": Weg[EPC * c:EPC * (c + 1)].astype(BF),
            "weu": Weu[EPC * c:EPC * (c + 1)].astype(BF),
            "wed": Wed[EPC * c:EPC * (c + 1)].astype(BF),
        })

    nc2 = _get_nc(("moe", c_cap), build_moe, c_cap)
    res2 = run_bass_kernel_spmd(nc2, in_maps2, core_ids=list(range(N_CORES)),
                                trace=_profile is not None)

    out = h1
    for c in range(N_CORES):
        cb = res2.results[c]["contrib"].reshape(EPC, c_cap, H)
        for j in range(EPC):
            e = EPC * c + j
            ix = idxs[e]
            out[ix] += cb[j, :len(ix)].astype(np.float32)

    if _profile is not None:
        _profile["attn_ns"] = res1.exec_time_ns
        _profile["moe_ns"] = res2.exec_time_ns
        _profile["res1"] = res1
        _profile["res2"] = res2

    return out.reshape(b, s, H)


# revision 18
# speedup vs baseline: 1.4531x; 1.0813x over previous
"""Trainium2 Bass kernel for nn_HFMiMoV2DecoderLayer (attention + MoE decoder layer).

Strategy (8 NeuronCores):
  Launch 1 — tensor-parallel attention: each core owns 2 of 16 heads (and the
    matching GQA KV head). Host pre-transposes x and pre-computes per-token
    RMS scales, so the device does no rmsnorm/transpose of the full x.
    QKV is one merged [H, 512] matmul per core; rope is 4 strided-AP vector
    ops per token chunk; flash-style causal sink-softmax with ln/exp
    denominator; partial Wo product [T, H] written per core.
  Host    — h1 = x + sum(partials); exact MoE routing (numpy, mirrors the
    reference); builds per-expert gathered activation matrices.
  Launch 2 — expert-parallel MoE FF in bf16 (post-gate path is precision-
    safe): each core owns 2 of 16 experts; combine weight folded into the
    PSUM->SBUF output copy.
  Host    — scatter-add contributions into h1.

The h1/routing path stays fp32 end-to-end: min routing margin for this
layer's data is ~3e-5; perturbation tests show bf16 anywhere before the gate
risks a top-k flip costing ~1.4e-1 rel err. Post-gate bf16 measures 1.2e-3.
"""
import sys
import types

import numpy as np


def _install_ntff_hook():
    """bass_utils needs antenv.axon_hooks for NTFF tracing under axon; the
    image's antenv lacks that submodule. Inject a shim wired to the ctypes
    hook from trn_agent_boot (no-op if anything is missing)."""
    if "antenv.axon_hooks" in sys.modules:
        return
    try:
        from trn_agent_boot.trn_boot import _ntff_profile_via_ctypes

        hook = _ntff_profile_via_ctypes("/opt/axon/libaxon_pjrt.so")
    except Exception:
        hook = None
    mod = types.ModuleType("antenv.axon_hooks")
    mod._hook = hook
    mod.set_axon_ntff_profile_hook = lambda h: setattr(mod, "_hook", h)
    mod.get_axon_ntff_profile_hook = lambda: mod._hook
    sys.modules["antenv.axon_hooks"] = mod


_install_ntff_hook()

import ml_dtypes

import concourse.bass as bass
import concourse.mybir as mybir
import concourse.tile as tile
from concourse import bacc
from concourse.bass_utils import run_bass_kernel_spmd
from concourse.masks import make_identity

F32 = mybir.dt.float32
F32R = mybir.dt.float32r
BF16 = mybir.dt.bfloat16
BF = ml_dtypes.bfloat16

N_CORES = 8
T = 2048          # tokens
H = 2048          # hidden
P = 128
TCH = T // P      # 16 token chunks
HCH = H // P      # 16 hidden chunks
HD = 128          # head dim
NHC = 2           # heads per core
RD = 64           # rope dims
RH = 32
FF = 512          # moe intermediate
FFC = FF // P     # 4
E = 16
EPC = 2           # experts per core
SCALE = HD ** -0.5
EPS = 1e-6
ROUTE_SCALE = 2.5
G, TG, TK = 4, 2, 4

QG = 512          # query-group width for attention
NQG = T // QG     # 4
TGRP = 4          # token chunks loaded per DMA in phase A


def _r32(ap):
    return ap.bitcast(F32R)


def _mk_nc():
    return bacc.Bacc("TRN2", target_bir_lowering=False, debug=False,
                     num_devices=N_CORES)


# --------------------------------------------------------------------------
# Launch 1: attention (2 heads per core)
# --------------------------------------------------------------------------

def build_attn():
    nc = _mk_nc()
    xt = nc.dram_tensor("xt", [H, T], F32R, kind="ExternalInput")
    wqkv = nc.dram_tensor("wqkv", [H, 4 * P], F32R, kind="ExternalInput")
    rsc = nc.dram_tensor("rsc", [P, TCH], F32, kind="ExternalInput")
    cosr = nc.dram_tensor("cosr", [P, TCH, RD], F32, kind="ExternalInput")
    sinr = nc.dram_tensor("sinr", [P, TCH, RD], F32, kind="ExternalInput")
    sinke = nc.dram_tensor("sinke", [1, NHC], F32, kind="ExternalInput")
    wo = nc.dram_tensor("wo", [NHC * HD, H], F32R, kind="ExternalInput")
    partial = nc.dram_tensor("partial", [T, H], F32, kind="ExternalOutput")

    xt_r = xt.rearrange("(hc p) t -> p hc t", p=P)
    pt_out = partial.rearrange("(tc p) h -> tc p h", p=P)

    with tile.TileContext(nc) as tc:
        with (
            tc.tile_pool(name="persist", bufs=1) as pers,
            tc.tile_pool(name="const", bufs=1) as constp,
        ):
            rsc_s = pers.tile([P, TCH], F32)
            nc.sync.dma_start(rsc_s[:], rsc[:])
            sinke_s = pers.tile([1, NHC], F32)
            nc.sync.dma_start(sinke_s[:], sinke[:])
            # wqkv/cos/sin DMAs are interleaved with the first xt chunk's
            # loads inside phase A so the first QKV matmuls gate on ~2MB
            wqkv_s = pers.tile([P, HCH, 4 * P], F32R)
            wqkv_r = wqkv.rearrange("(hc p) n -> p hc n", p=P)
            cos_s = pers.tile([P, TCH, RD], F32)
            sin_s = pers.tile([P, TCH, RD], F32)

            ident0 = constp.tile([P, P], F32)
            make_identity(nc, ident0[:])
            ident = constp.tile([P, P], F32R)
            nc.vector.tensor_copy(ident[:], ident0[:])
            ones0 = constp.tile([P, P], F32)
            nc.vector.memset(ones0[:], 1.0)
            ones_row = constp.tile([1, P], F32R)
            nc.vector.tensor_copy(ones_row[:], ones0[0:1, :])
            ones_col = constp.tile([P, 1], F32R)
            nc.vector.tensor_copy(ones_col[:], ones0[:, 0:1])

            # diagonal-block causal masks, built on the idle gpsimd engine:
            # mask[p, d, q] = 1.0 if q >= 128*d + p else 0.0
            mask_s = constp.tile([P, NQG, QG], F32)
            nc.gpsimd.memset(mask_s[:], 1.0)
            for d in range(4):
                nc.gpsimd.affine_select(
                    out=mask_s[:, d, :], in_=mask_s[:, d, :],
                    compare_op=mybir.AluOpType.is_ge, fill=0.0,
                    base=-(P * d), pattern=[[1, QG]], channel_multiplier=-1)

            qkv_sb = pers.tile([P, TCH, 4 * P], F32R)  # roped q0|q1|k|v per chunk
            qkt_s = pers.tile([P, 3, T], F32R)         # q0^T | q1^T | k^T
            ot_s = pers.tile([P, NHC, T], F32R)        # O^T  [hd, h, tok]

            # ---------------- phase A: QKV + rope + transposes ----------------
            with (
                tc.tile_pool(name="xin", bufs=2) as xpool,
                tc.tile_pool(name="rope", bufs=2) as ropep,
                tc.tile_pool(name="psA", bufs=2, space="PSUM") as psA,
                tc.tile_pool(name="psT", bufs=2, space="PSUM") as psT,
            ):
                def rope_and_transpose(tcx):
                    # rope on q0, q1, k via strided views [P, 3, 64]
                    qv = qkv_sb[:, tcx, :].rearrange("p (b c) -> p b c", c=P)
                    ro = qv[:, 0:3, 0:RD]
                    cos_b = cos_s[:, tcx:tcx + 1, :].broadcast_to([P, 3, RD])
                    sin_lo = sin_s[:, tcx:tcx + 1, 0:RH].broadcast_to([P, 3, RH])
                    sin_hi = sin_s[:, tcx:tcx + 1, RH:RD].broadcast_to([P, 3, RH])
                    b = ropep.tile([P, 3, RD], F32, tag="rb")
                    nc.vector.tensor_mul(b[:, :, 0:RH], qv[:, 0:3, RH:RD],
                                         sin_lo)
                    nc.vector.tensor_mul(b[:, :, RH:RD], qv[:, 0:3, 0:RH],
                                         sin_hi)
                    a = ropep.tile([P, 3, RD], F32, tag="ra")
                    nc.vector.tensor_mul(a[:], ro, cos_b)
                    nc.vector.tensor_add(ro, a[:], b[:])
                    # transpose q0, q1, k into qkt_s
                    ps_t = psT.tile([P, 3, P], F32R, tag="pt")
                    for i in range(3):
                        nc.tensor.transpose(ps_t[:, i, :], qv[:, i, :],
                                            ident[:])
                    nc.vector.tensor_copy(
                        qkt_s[:, :, tcx * P:(tcx + 1) * P], ps_t[:])

                pending = None
                for tg in range(TCH // TGRP):
                    xt_g = xpool.tile([P, HCH, TGRP * P], F32R, tag="xt")
                    tgs = slice(tg * TGRP * P, (tg + 1) * TGRP * P)
                    for q in range(4):
                        hs4 = slice(4 * q, 4 * (q + 1))
                        if tg == 0:
                            nc.sync.dma_start(wqkv_s[:, hs4, :],
                                              wqkv_r[:, hs4, :])
                        nc.sync.dma_start(xt_g[:, hs4, :], xt_r[:, hs4, tgs])
                        if tg == 0 and q == 1:
                            nc.sync.dma_start(cos_s[:], cosr[:])
                            nc.sync.dma_start(sin_s[:], sinr[:])
                    for j in range(TGRP):
                        tcx = tg * TGRP + j
                        ts = slice(j * P, (j + 1) * P)
                        ps_qkv = psA.tile([P, 4 * P], F32, tag="qkv")
                        for hc in range(HCH):
                            nc.tensor.matmul(ps_qkv[:], _r32(xt_g[:, hc, ts]),
                                             _r32(wqkv_s[:, hc, :]),
                                             start=(hc == 0), stop=(hc == HCH - 1))
                        nc.scalar.activation(qkv_sb[:, tcx, :], ps_qkv[:],
                                             mybir.ActivationFunctionType.Copy,
                                             scale=rsc_s[:, tcx:tcx + 1])
                        # rope/transpose of the PREVIOUS chunk: keeps the PE
                        # stream dense (transposes' deps are ready by now)
                        if pending is not None:
                            rope_and_transpose(pending)
                        pending = tcx
                rope_and_transpose(pending)

            # ---------------- phase B + C interleaved ----------------
            with (
                tc.tile_pool(name="wop", bufs=1) as wop,
                tc.tile_pool(name="psS", bufs=3, space="PSUM") as psS,
                tc.tile_pool(name="psO", bufs=2, space="PSUM") as psO,
                tc.tile_pool(name="psD", bufs=1, space="PSUM") as psD,
                tc.tile_pool(name="psX", bufs=2, space="PSUM") as psX,
                tc.tile_pool(name="ptp", bufs=4) as ptp,
                tc.tile_pool(name="den", bufs=2) as denp,
                tc.tile_pool(name="outp", bufs=3) as outp,
            ):
                wo_s = wop.tile([P, NHC, H], F32R)
                nc.sync.dma_start(wo_s[:], wo.rearrange("(h p) n -> p h n", p=P))

                def phase_c(qg):
                    for ti in range(4 * qg, 4 * (qg + 1)):
                        out_sb = outp.tile([P, H], F32, tag="osb")
                        for nt in range(H // 512):
                            ps_p = psX.tile([P, 512], F32, tag="psx")
                            for h in range(NHC):
                                nc.tensor.matmul(
                                    ps_p[:],
                                    _r32(ot_s[:, h, ti * P:(ti + 1) * P]),
                                    _r32(wo_s[:, h, nt * 512:(nt + 1) * 512]),
                                    start=(h == 0), stop=(h == NHC - 1))
                            dst = out_sb[:, nt * 512:(nt + 1) * 512]
                            if nt % 2 == 0:
                                nc.scalar.activation(
                                    dst, ps_p[:],
                                    mybir.ActivationFunctionType.Copy)
                            else:
                                nc.vector.tensor_copy(dst, ps_p[:])
                        nc.sync.dma_start(pt_out[ti], out_sb[:])

                for qg in range(NQG):
                    nkt = 4 * (qg + 1)
                    # diagonal (masked) blocks first: their longer
                    # exp+mask chains get covered by the score lookahead
                    kt_order = list(range(4 * qg, nkt)) + list(range(4 * qg))
                    for h in range(NHC):
                        q_rhs = _r32(qkt_s[:, h, qg * QG:(qg + 1) * QG])
                        ps_o = psO.tile([P, QG], F32, tag="pso")
                        ps_d = psD.tile([1, QG], F32, tag="psd")
                        pts = {}

                        def score(i):
                            kt = kt_order[i]
                            ps_s = psS.tile([P, QG], F32, tag="pss")
                            nc.tensor.matmul(
                                ps_s[:],
                                _r32(qkt_s[:, 2, kt * P:(kt + 1) * P]),
                                q_rhs, start=True, stop=True)
                            p_t = ptp.tile([P, QG], F32R, tag="pt")
                            nc.scalar.activation(
                                p_t[:], ps_s[:],
                                mybir.ActivationFunctionType.Exp, scale=SCALE)
                            d = kt - 4 * qg
                            if d >= 0:  # diagonal block: causal mask, only
                                w = (d + 1) * P  # cols < w can be acausal
                                nc.vector.tensor_mul(p_t[:, :w], p_t[:, :w],
                                                     mask_s[:, d, :w])
                            pts[i] = p_t

                        def accum(i):
                            kt = kt_order[i]
                            p_t = pts.pop(i)
                            nc.tensor.matmul(
                                ps_o[:], _r32(qkv_sb[:, kt, 3 * P:4 * P]),
                                _r32(p_t[:]),
                                start=(i == 0), stop=(i == nkt - 1))
                            nc.tensor.matmul(
                                ps_d[:], ones_col[:], _r32(p_t[:]),
                                start=(i == 0), stop=(i == nkt - 1))

                        score(0)
                        if nkt > 1:
                            score(1)
                        for i in range(nkt):
                            if i + 2 < nkt:
                                score(i + 2)
                            accum(i)
                        den = denp.tile([1, QG], F32, tag="den")
                        nc.vector.tensor_scalar(den[:], ps_d[:],
                                                sinke_s[0:1, h:h + 1], None,
                                                mybir.AluOpType.add)
                        rec = denp.tile([1, QG], F32R, tag="rec")
                        with nc.allow_low_precision(reason="f32r rhs for PE"):
                            nc.vector.reciprocal(rec[:], den[:])
                        ps_bc = psX.tile([P, QG], F32, tag="psx")
                        nc.tensor.matmul(ps_bc[:], ones_row[:], _r32(rec[:]),
                                         start=True, stop=True)
                        rbc = denp.tile([P, QG], F32, tag="rbc")
                        nc.vector.tensor_copy(rbc[:], ps_bc[:])
                        nc.vector.tensor_mul(ot_s[:, h, qg * QG:(qg + 1) * QG],
                                             ps_o[:], rbc[:])
                    # phase C for the PREVIOUS query group: its ot_s values
                    # are long since ready, so the PE stream never stalls
                    if qg >= 1:
                        phase_c(qg - 1)
                phase_c(NQG - 1)

    nc.finalize()
    return nc


# --------------------------------------------------------------------------
# Launch 2: MoE expert FF in bf16 (2 experts per core, capacity c_cap each)
# --------------------------------------------------------------------------

def _n_chunks(c):
    """Split c into moving-dim chunks, each in [256, 512] (c >= 512)."""
    out = []
    rem = c
    while rem > 512:
        take = 512 if rem - 512 >= 256 or rem - 512 == 0 else 384
        out.append(take)
        rem -= take
    out.append(rem)
    assert all(128 <= n <= 512 for n in out) and sum(out) == c, (c, out)
    return out


def build_moe(c_cap):
    nc = _mk_nc()
    cch = c_cap // P
    xgt = nc.dram_tensor("xgt", [EPC, H, c_cap], BF16, kind="ExternalInput")
    wrow = nc.dram_tensor("wrow", [P, EPC, cch], F32, kind="ExternalInput")
    weg = nc.dram_tensor("weg", [EPC, H, FF], BF16, kind="ExternalInput")
    weu = nc.dram_tensor("weu", [EPC, H, FF], BF16, kind="ExternalInput")
    wed = nc.dram_tensor("wed", [EPC, FF, H], BF16, kind="ExternalInput")
    contrib = nc.dram_tensor("contrib", [EPC * c_cap, H], BF16,
                             kind="ExternalOutput")
    co = contrib.rearrange("(ec tc p) h -> ec tc p h", p=P, ec=EPC)

    nch = _n_chunks(c_cap)
    ncn = len(nch)

    with tile.TileContext(nc) as tc:
        with (
            tc.tile_pool(name="wr", bufs=1) as wrp,
            tc.tile_pool(name="xg", bufs=2) as xgp,
            tc.tile_pool(name="wgu", bufs=6) as wgup,
            tc.tile_pool(name="wd", bufs=2) as wdp,
            tc.tile_pool(name="hgu", bufs=2) as hgup,
            tc.tile_pool(name="act", bufs=3) as actp,
            tc.tile_pool(name="outp", bufs=3) as outp,
            tc.tile_pool(name="psGU", bufs=2, space="PSUM") as psGU,
            tc.tile_pool(name="psC", bufs=2, space="PSUM") as psC,
        ):
            wr_s = wrp.tile([P, EPC, cch], F32)
            nc.sync.dma_start(wr_s[:], wrow[:])

            for e in range(EPC):
                xg_s = xgp.tile([P, HCH, c_cap], BF16, tag="xg")
                xg_r = xgt[e].rearrange("(hc p) c -> p hc c", p=P)
                wd_s = wdp.tile([P, FFC, H], BF16, tag="wd")

                hgu = hgup.tile([P, FFC, c_cap], BF16, tag="hgu")
                for fc in range(FFC):
                    wg_s = wgup.tile([P, HCH, P], BF16, tag="wg")
                    nc.sync.dma_start(
                        wg_s[:],
                        weg[e].rearrange("(hc p) f -> p hc f", p=P)[
                            :, :, fc * P:(fc + 1) * P])
                    if fc == 0:
                        nc.sync.dma_start(xg_s[:, 0:4, :], xg_r[:, 0:4, :])
                    wu_s = wgup.tile([P, HCH, P], BF16, tag="wu")
                    nc.sync.dma_start(
                        wu_s[:],
                        weu[e].rearrange("(hc p) f -> p hc f", p=P)[
                            :, :, fc * P:(fc + 1) * P])
                    if fc == 0:
                        for xq in range(1, 4):
                            nc.sync.dma_start(xg_s[:, 4 * xq:4 * (xq + 1), :],
                                              xg_r[:, 4 * xq:4 * (xq + 1), :])
                    if fc == 2:
                        # down-proj weights: needed only after gate/up
                        nc.sync.dma_start(
                            wd_s[:], wed[e].rearrange("(fc p) h -> p fc h",
                                                      p=P))
                    nco = 0
                    for j, nsz in enumerate(nch):
                        cs = slice(nco, nco + nsz)
                        ps_g = psGU.tile([P, 512], F32, tag="psg")
                        for hc in range(HCH):
                            nc.tensor.matmul(ps_g[:, :nsz], wg_s[:, hc, :],
                                             xg_s[:, hc, cs],
                                             start=(hc == 0),
                                             stop=(hc == HCH - 1))
                        ps_u = psGU.tile([P, 512], F32, tag="psu")
                        for hc in range(HCH):
                            nc.tensor.matmul(ps_u[:, :nsz], wu_s[:, hc, :],
                                             xg_s[:, hc, cs],
                                             start=(hc == 0),
                                             stop=(hc == HCH - 1))
                        sg = actp.tile([P, 512], F32, tag="sg")
                        nc.scalar.activation(sg[:, :nsz], ps_g[:, :nsz],
                                             mybir.ActivationFunctionType.Silu)
                        nc.vector.tensor_mul(hgu[:, fc, cs],
                                             sg[:, :nsz], ps_u[:, :nsz])
                        nco += nsz

                # down projection, combine weight folded into the output copy
                for ti in range(cch):
                    out_sb = outp.tile([P, H], BF16, tag="osb")
                    wr_ap = wr_s[:, e, ti:ti + 1]
                    for ntg in range(2):
                        ps_c0 = psC.tile([P, 512], F32, tag="psc0")
                        ps_c1 = psC.tile([P, 512], F32, tag="psc1")
                        for fc in range(FFC):
                            for k, ps_c in enumerate((ps_c0, ps_c1)):
                                nt = 2 * ntg + k
                                nc.tensor.matmul(
                                    ps_c[:],
                                    hgu[:, fc, ti * P:(ti + 1) * P],
                                    wd_s[:, fc, nt * 512:(nt + 1) * 512],
                                    start=(fc == 0), stop=(fc == FFC - 1))
                        for k, ps_c in enumerate((ps_c0, ps_c1)):
                            nt = 2 * ntg + k
                            dst = out_sb[:, nt * 512:(nt + 1) * 512]
                            if k == 0:
                                nc.scalar.activation(
                                    dst, ps_c[:],
                                    mybir.ActivationFunctionType.Copy,
                                    scale=wr_ap)
                            else:
                                nc.vector.tensor_scalar(
                                    dst, ps_c[:], wr_ap, None,
                                    mybir.AluOpType.mult)
                    nc.sync.dma_start(co[e, ti], out_sb[:])

    nc.finalize()
    return nc


# --------------------------------------------------------------------------
# Host-side routing (numpy mirror of the reference MoE gate)
# --------------------------------------------------------------------------

def _routing(h1, ln2_w, gate_w, gate_bias):
    var = np.mean(h1 * h1, axis=-1, keepdims=True)
    xf = (ln2_w * (h1 / np.sqrt(var + EPS))).astype(np.float32)
    logits = xf @ gate_w.T
    s = 1.0 / (1.0 + np.exp(-logits))
    sfc = s + gate_bias[None]
    n = sfc.shape[0]
    gview = sfc.reshape(n, G, E // G)
    gsort = np.sort(gview, axis=-1)
    group_scores = gsort[..., -1] + gsort[..., -2]
    gidx = np.argsort(-group_scores, kind="stable", axis=-1)[:, :TG]
    gmask = np.zeros((n, G), np.bool_)
    np.put_along_axis(gmask, gidx, True, axis=1)
    smask = np.repeat(gmask, E // G, axis=1)
    tmp = np.where(smask, sfc, -np.inf)
    tidx = np.argsort(-tmp, kind="stable", axis=-1)[:, :TK]
    tw = np.take_along_axis(s, tidx, axis=1)
    tw = tw / (tw.sum(-1, keepdims=True) + 1e-20)
    tw = tw * ROUTE_SCALE
    cw = np.zeros((n, E), np.float32)
    np.put_along_axis(cw, tidx, tw.astype(np.float32), axis=1)
    return xf, cw


# --------------------------------------------------------------------------
# Entry point
# --------------------------------------------------------------------------

_NC_CACHE = {}


def _get_nc(key, builder, *args):
    if key not in _NC_CACHE:
        _NC_CACHE[key] = builder(*args)
    return _NC_CACHE[key]


def kernel(hidden_states, cos, sin, ln1_w, ln2_w, Wq, Wk, Wv, Wo,
           sink_bias, gate_w, gate_bias, Weg, Weu, Wed, _profile=None):
    hidden_states, cos, sin, ln1_w, ln2_w = map(
        np.asarray, (hidden_states, cos, sin, ln1_w, ln2_w))
    Wq, Wk, Wv, Wo, sink_bias = map(np.asarray, (Wq, Wk, Wv, Wo, sink_bias))
    gate_w, gate_bias, Weg, Weu, Wed = map(
        np.asarray, (gate_w, gate_bias, Weg, Weu, Wed))
    b, s, _ = hidden_states.shape
    x = np.ascontiguousarray(hidden_states.reshape(T, H), dtype=np.float32)
    cosb = np.ascontiguousarray(cos.reshape(T, RD), dtype=np.float32)
    sinb = np.ascontiguousarray(sin.reshape(T, RD), dtype=np.float32)

    # host-side prep: transpose of x, per-token 1/rms, rope tables
    xt = np.ascontiguousarray(x.T)
    r = (1.0 / np.sqrt((x * x).mean(-1) + EPS)).astype(np.float32)
    rsc = np.ascontiguousarray(r.reshape(TCH, P).T)
    cosr = np.ascontiguousarray(cosb.reshape(TCH, P, RD).transpose(1, 0, 2))
    ss = sinb.copy()
    ss[:, :RH] *= -1.0
    sinr = np.ascontiguousarray(ss.reshape(TCH, P, RD).transpose(1, 0, 2))

    # fold ln1 into the QKV weights
    wq_f = (ln1_w[:, None] * Wq).astype(np.float32)
    wk_f = (ln1_w[:, None] * Wk).astype(np.float32)
    wv_f = (ln1_w[:, None] * Wv).astype(np.float32)

    in_maps = []
    for c in range(N_CORES):
        h0 = NHC * c
        g0 = h0 // (16 // 4)  # kv head
        in_maps.append({
            "xt": xt,
            "wqkv": np.ascontiguousarray(np.concatenate(
                [wq_f[:, h0 * HD:(h0 + NHC) * HD],
                 wk_f[:, g0 * HD:(g0 + 1) * HD],
                 wv_f[:, g0 * HD:(g0 + 1) * HD]], axis=1)),
            "rsc": rsc,
            "cosr": cosr,
            "sinr": sinr,
            "sinke": np.exp(sink_bias[h0:h0 + NHC]).reshape(1, NHC)
                       .astype(np.float32),
            "wo": np.ascontiguousarray(Wo[h0 * HD:(h0 + NHC) * HD, :]),
        })

    nc1 = _get_nc("attn", build_attn)
    res1 = run_bass_kernel_spmd(nc1, in_maps, core_ids=list(range(N_CORES)),
                                trace=_profile is not None)
    h1 = x.copy()
    for c in range(N_CORES):
        h1 += res1.results[c]["partial"]

    xf, cw = _routing(h1, np.asarray(ln2_w), np.asarray(gate_w),
                      np.asarray(gate_bias))

    idxs = [np.nonzero(cw[:, e] > 0)[0] for e in range(E)]
    maxc = max(len(ix) for ix in idxs)
    c_cap = max(512, -(-maxc // P) * P)
    cch = c_cap // P

    in_maps2 = []
    for c in range(N_CORES):
        xg = np.zeros((EPC, H, c_cap), BF)
        wr = np.zeros((EPC, c_cap), np.float32)
        for j in range(EPC):
            e = EPC * c + j
            ix = idxs[e]
            xg[j, :, :len(ix)] = xf[ix].T.astype(BF)
            wr[j, :len(ix)] = cw[ix, e]
        in_maps2.append({
            "xgt": xg,
            "wrow": np.ascontiguousarray(
                wr.reshape(EPC, cch, P).transpose(2, 0, 1)),
            "weg": Weg[EPC * c:EPC * (c + 1)].astype(BF),
            "weu": Weu[EPC * c:EPC * (c + 1)].astype(BF),
            "wed": Wed[EPC * c:EPC * (c + 1)].astype(BF),
        })

    nc2 = _get_nc(("moe", c_cap), build_moe, c_cap)
    res2 = run_bass_kernel_spmd(nc2, in_maps2, core_ids=list(range(N_CORES)),
                                trace=_profile is not None)

    out = h1
    for c in range(N_CORES):
        cb = res2.results[c]["contrib"].reshape(EPC, c_cap, H)
        for j in range(EPC):
            e = EPC * c + j
            ix = idxs[e]
            out[ix] += cb[j, :len(ix)].astype(np.float32)

    if _profile is not None:
        _profile["attn_ns"] = res1.exec_time_ns
        _profile["moe_ns"] = res2.exec_time_ns
        _profile["res1"] = res1
        _profile["res2"] = res2

    return out.reshape(b, s, H)
